# revision 3
# baseline (speedup 1.0000x reference)
"""Trainium2 Bass kernel for sorted segment_max (ClusterPool).

Problem: features [2M, 128] f32, segment_ids [2M] sorted int, num_clusters
10000 -> output [10000, 128] = per-cluster max over rows.

Strategy (8 NeuronCores, SPMD):
  - Shard rows: core c gets rows [c*250k, (c+1)*250k). Sorted ids => each
    core covers a contiguous cluster range (~1252 clusters), padded to 1280
    local clusters = 10 batches x 128.
  - Host precomputes, per core, per batch of 128 clusters, an int16 gather
    index table: cluster p's rows (padded by repeating its first row to a
    fixed slot count L). Indices are relative to a per-batch 32768-row
    window of the core's feature slice (clusters are contiguous, so every
    batch's rows fit in one window; verified at runtime).
  - Device: per batch, dma_gather rows into SBUF [128, L, 128] (cluster p on
    partition p), tensor_reduce max over the slot axis -> [128, 128], store
    to the partial output rows. Two gathers of L/2 slots per batch to halve
    SBUF footprint.
  - Host combines the 8 partial outputs (adjacent cores overlap in at most
    one boundary cluster) with np.maximum.
"""

import os
import sys

import numpy as np

sys.path.insert(0, "/opt/trn_rl_repo")

N_POINTS = 2_000_000
D = 128
N_CLUSTERS = 10_000
N_CORES = 8
RPC = N_POINTS // N_CORES  # rows per core
NCL = 1280  # padded local clusters per core
NBATCH = NCL // 128
WINDOW = 32768  # dma_gather int16 index window (rows)

_last_results = None  # BassKernelResults of the most recent run (for test.py)


def _apply_drain_patch():
    """walrus TPB_CTRL supports a single sync wait; TileContext's tail drain
    accumulates one wait per outstanding proc.  Split them across NOPs."""
    import concourse.mybir as mybir
    import concourse.tile as tile
    from concourse.vector_clock import ScopedClock

    if getattr(tile.TileContext, "_drain_patched", False):
        return

    def _patched(self, tick_clock, wait_clock):
        nc = self.nc
        nop = nc.sync.nop(nofuse=True, hint="tail_drain_waits")
        wait_clock.add_sem_waits(nop.ins, ScopedClock({None: tick_clock.global_clock}))
        si = nop.ins.sync_info
        waits = list(si.on_wait) if si is not None and si.on_wait else []
        if len(waits) > 1:
            si.on_wait = waits[:1]
            for i in range(1, len(waits)):
                extra = nc.sync.nop(nofuse=True, hint=f"tail_drain_waits_{i}")
                if extra.ins.sync_info is None:
                    extra.ins.sync_info = mybir.SyncInfo(
                        on_wait=waits[i : i + 1], on_update=[]
                    )
                else:
                    extra.ins.sync_info.on_wait = waits[i : i + 1]
        nc.sync.drain()
        nc.all_engine_barrier()
        assert self.sems is not None
        popped = nc._tile_sem_poison_stack.pop()
        assert popped is self._sem_poison
        nc.clear_and_free_semaphores(list(self.sems.allocated().values()))
        nc.all_engine_barrier()

    tile.TileContext._drain_and_barrier = _patched
    tile.TileContext._drain_patched = True


def _build_program(LH, windows):
    """Build the SPMD Bass program.  LH = slots per half-gather; windows =
    per-batch window base rows (compile-time constants, shared by all cores)."""
    import concourse.bacc as bacc
    import concourse.mybir as mybir
    import concourse.tile as tile

    _apply_drain_patch()

    NG = LH // 8  # gathers per half-batch (1024 idx each: 8 slots/partition)
    IW = 64  # idx cols per gather (1024/16)

    nc = bacc.Bacc(None)
    f_in = nc.dram_tensor("features", [RPC, D], mybir.dt.float32, kind="ExternalInput")
    i_in = nc.dram_tensor(
        "gidx", [NBATCH, 2, 128, NG * IW], mybir.dt.int16, kind="ExternalInput"
    )
    p_out = nc.dram_tensor(
        "partial", [NCL, D], mybir.dt.float32, kind="ExternalOutput"
    )

    with tile.TileContext(nc) as tc:
        with (
            tc.tile_pool(name="gp", bufs=2) as gp,
            tc.tile_pool(name="sp", bufs=3) as sp,
        ):
            for b in range(NBATCH):
                w = windows[b]
                halves = []
                for h in range(2):
                    it = sp.tile([128, NG * IW], mybir.dt.int16, tag="idx")
                    nc.sync.dma_start(out=it[:], in_=i_in[b, h])
                    g = gp.tile([128, LH * D], mybir.dt.float32, tag="gath")
                    for k in range(NG):
                        nc.gpsimd.dma_gather(
                            out_ap=g[:, k * 8 * D : (k + 1) * 8 * D].rearrange(
                                "p (t d) -> p t d", d=D
                            ),
                            in_ap=f_in[w : w + WINDOW, :],
                            idxs_ap=it[:, k * IW : (k + 1) * IW],
                            num_idxs=1024,
                            num_idxs_reg=1024,
                            elem_size=D,
                        )
                    r = sp.tile([128, D], mybir.dt.float32, tag=f"red{h}")
                    nc.vector.tensor_reduce(
                        out=r[:],
                        in_=g[:].rearrange("p (t d) -> p d t", d=D),
                        axis=mybir.AxisListType.X,
                        op=mybir.AluOpType.max,
                    )
                    halves.append(r)
                o = sp.tile([128, D], mybir.dt.float32, tag="out")
                nc.vector.tensor_tensor(
                    out=o[:],
                    in0=halves[0][:],
                    in1=halves[1][:],
                    op=mybir.AluOpType.max,
                )
                nc.sync.dma_start(out=p_out[b * 128 : (b + 1) * 128, :], in_=o[:])

    if not nc.is_finalized():
        nc.finalize()
    return nc


def kernel(features, segment_ids, num_clusters):
    global _last_results
    from concourse.bass_utils import run_bass_kernel_spmd

    features = np.ascontiguousarray(np.asarray(features, dtype=np.float32))
    ids = np.asarray(segment_ids).astype(np.int64)
    nclusters = int(num_clusters)
    assert features.shape == (N_POINTS, D), features.shape
    assert ids.shape == (N_POINTS,)
    assert nclusters == N_CLUSTERS

    # --- host index prep -------------------------------------------------
    gstart = np.searchsorted(ids, np.arange(nclusters), side="left")
    gend = np.searchsorted(ids, np.arange(nclusters) + 1, side="left")
    gcounts = gend - gstart

    core_meta = []  # (cl_lo, ncl, s[NCL], cnt[NCL]) per core
    for c in range(N_CORES):
        r0, r1 = c * RPC, (c + 1) * RPC
        cl_lo, cl_hi = int(ids[r0]), int(ids[r1 - 1])
        ncl = cl_hi - cl_lo + 1
        assert ncl <= NCL, f"core {c}: {ncl} local clusters > {NCL}"
        s = np.clip(gstart[cl_lo : cl_hi + 1], r0, r1) - r0
        e = np.clip(gend[cl_lo : cl_hi + 1], r0, r1) - r0
        cnt = e - s
        s_pad = np.full(NCL, RPC - 1, dtype=np.int64)
        cnt_pad = np.zeros(NCL, dtype=np.int64)
        s_pad[:ncl] = s
        cnt_pad[:ncl] = cnt
        core_meta.append((cl_lo, ncl, s_pad, cnt_pad))

    L = max(int(m[3].max()) for m in core_meta)
    LH = (L + 1) // 2
    LH = max(8, (LH + 7) // 8 * 8)  # pad slots-per-half to a multiple of 8

    # Per-batch windows, shared across cores.
    windows = []
    for b in range(NBATCH):
        wmin = min(int(m[2][b * 128]) for m in core_meta)
        w = max(0, min(wmin, RPC - WINDOW))
        for m in core_meta:
            s_pad, cnt_pad = m[2], m[3]
            jj = slice(b * 128, (b + 1) * 128)
            last = s_pad[jj] + np.maximum(cnt_pad[jj] - 1, 0)
            active = cnt_pad[jj] > 0
            assert not active.any() or (
                (s_pad[jj][active] >= w).all()
                and (last[active] < w + WINDOW).all()
            ), f"batch {b}: window overflow"
        windows.append(w)

    # Build int16 gather tables: gidx[core][b, h, 128, NIDX//16]
    slots = np.arange(2 * LH)
    gidx_all = []
    for c in range(N_CORES):
        _, _, s_pad, cnt_pad = core_meta[c]
        # rows[j, s] = s_j + min(s, cnt_j - 1)   (cnt 0 -> window base)
        rows = s_pad[:, None] + np.minimum(slots[None, :], np.maximum(cnt_pad - 1, 0)[:, None])
        offs = np.empty((NBATCH, 128, 2 * LH), dtype=np.int64)
        for b in range(NBATCH):
            o = rows[b * 128 : (b + 1) * 128] - windows[b]
            o[cnt_pad[b * 128 : (b + 1) * 128] == 0] = 0
            offs[b] = o
        assert offs.min() >= 0 and offs.max() < WINDOW, (offs.min(), offs.max())
        offs16 = offs.astype(np.int16)
        # half h slots [h*LH,(h+1)*LH); gather j-index = t*128+p -> [j%16, j//16]
        NG = LH // 8
        g = np.empty((NBATCH, 2, 128, NG * 64), dtype=np.int16)
        for b in range(NBATCH):
            for h in range(2):
                A = offs16[b, :, h * LH : (h + 1) * LH]  # [128 p, LH]
                # gather k covers slots [k*8,(k+1)*8); flat j = t*128+p
                A = A.reshape(128, NG, 8).transpose(1, 2, 0).reshape(NG, 1024)
                W = A.reshape(NG, 64, 16).transpose(0, 2, 1)  # wrap [j%16, j//16]
                W = W.transpose(1, 0, 2).reshape(16, NG * 64)
                g[b, h] = np.tile(W, (8, 1))
        gidx_all.append(g)

    # --- build + run ------------------------------------------------------
    nc = _build_program(LH, windows)
    in_maps = [
        {
            "features": features[c * RPC : (c + 1) * RPC],
            "gidx": gidx_all[c],
        }
        for c in range(N_CORES)
    ]
    res = run_bass_kernel_spmd(nc, in_maps, list(range(N_CORES)))
    _last_results = res

    # --- host combine -----------------------------------------------------
    full = np.full((nclusters, D), -np.inf, dtype=np.float32)
    for c in range(N_CORES):
        cl_lo, ncl, _, _ = core_meta[c]
        part = res.results[c]["partial"][:ncl]
        full[cl_lo : cl_lo + ncl] = np.maximum(full[cl_lo : cl_lo + ncl], part)
    full[gcounts == 0] = -np.inf
    return full


# revision 4
# speedup vs baseline: 2.8093x; 2.8093x over previous
"""Trainium2 Bass kernel for sorted segment_max (ClusterPool).

Problem: features [2M, 128] f32, segment_ids [2M] sorted int, num_clusters
10000 -> output [10000, 128] = per-cluster max over rows.

Strategy (8 NeuronCores, SPMD):
  - Shard rows: core c gets rows [c*250k, (c+1)*250k). Sorted ids => each
    core covers a contiguous cluster range (~1252 clusters), padded to 1280
    local clusters = 10 batches x 128.
  - Host precomputes, per core, per batch of 128 clusters, an int16 gather
    index table: cluster p's rows (padded by repeating its first row to a
    fixed slot count L). Indices are relative to a per-batch 32768-row
    window of the core's feature slice (clusters are contiguous, so every
    batch's rows fit in one window; verified at runtime).
  - Device: per batch, dma_gather rows into SBUF [128, L, 128] (cluster p on
    partition p), tensor_reduce max over the slot axis -> [128, 128], store
    to the partial output rows. Two gathers of L/2 slots per batch to halve
    SBUF footprint.
  - Host combines the 8 partial outputs (adjacent cores overlap in at most
    one boundary cluster) with np.maximum.
"""

import os
import sys

import numpy as np

sys.path.insert(0, "/opt/trn_rl_repo")

N_POINTS = 2_000_000
D = 128
N_CLUSTERS = 10_000
N_CORES = 8
RPC = N_POINTS // N_CORES  # rows per core
NCL = 1280  # padded local clusters per core
NBATCH = NCL // 128
WINDOW = 32768  # dma_gather int16 index window (rows)

_last_results = None  # BassKernelResults of the most recent run (for test.py)


def _apply_drain_patch():
    """walrus TPB_CTRL supports a single sync wait; TileContext's tail drain
    accumulates one wait per outstanding proc.  Split them across NOPs."""
    import concourse.mybir as mybir
    import concourse.tile as tile
    from concourse.vector_clock import ScopedClock

    if getattr(tile.TileContext, "_drain_patched", False):
        return

    def _patched(self, tick_clock, wait_clock):
        nc = self.nc
        nop = nc.sync.nop(nofuse=True, hint="tail_drain_waits")
        wait_clock.add_sem_waits(nop.ins, ScopedClock({None: tick_clock.global_clock}))
        si = nop.ins.sync_info
        waits = list(si.on_wait) if si is not None and si.on_wait else []
        if len(waits) > 1:
            si.on_wait = waits[:1]
            for i in range(1, len(waits)):
                extra = nc.sync.nop(nofuse=True, hint=f"tail_drain_waits_{i}")
                if extra.ins.sync_info is None:
                    extra.ins.sync_info = mybir.SyncInfo(
                        on_wait=waits[i : i + 1], on_update=[]
                    )
                else:
                    extra.ins.sync_info.on_wait = waits[i : i + 1]
        nc.sync.drain()
        nc.all_engine_barrier()
        assert self.sems is not None
        popped = nc._tile_sem_poison_stack.pop()
        assert popped is self._sem_poison
        nc.clear_and_free_semaphores(list(self.sems.allocated().values()))
        nc.all_engine_barrier()

    tile.TileContext._drain_and_barrier = _patched
    tile.TileContext._drain_patched = True


def _build_program(LH, windows):
    """Build the SPMD Bass program.  LH = slots per half-gather; windows =
    per-batch window base rows (compile-time constants, shared by all cores)."""
    import concourse.bacc as bacc
    import concourse.mybir as mybir
    import concourse.tile as tile

    _apply_drain_patch()

    NG = LH // 8  # gathers per half-batch (1024 idx each: 8 slots/partition)
    IW = 64  # idx cols per gather (1024/16)

    nc = bacc.Bacc(None, num_swdge_queues=4)
    f_in = nc.dram_tensor("features", [RPC, D], mybir.dt.float32, kind="ExternalInput")
    i_in = nc.dram_tensor(
        "gidx", [NBATCH, 2, 128, NG * IW], mybir.dt.int16, kind="ExternalInput"
    )
    p_out = nc.dram_tensor(
        "partial", [NCL, D], mybir.dt.float32, kind="ExternalOutput"
    )

    with tile.TileContext(nc) as tc:
        with (
            tc.tile_pool(name="gp", bufs=2) as gp,
            tc.tile_pool(name="sp", bufs=3) as sp,
        ):
            for b in range(NBATCH):
                w = windows[b]
                halves = []
                for h in range(2):
                    it = sp.tile([128, NG * IW], mybir.dt.int16, tag="idx")
                    nc.sync.dma_start(out=it[:], in_=i_in[b, h])
                    g = gp.tile([128, LH * D], mybir.dt.float32, tag="gath")
                    for k in range(NG):
                        nc.gpsimd.dma_gather(
                            out_ap=g[:, k * 8 * D : (k + 1) * 8 * D].rearrange(
                                "p (t d) -> p t d", d=D
                            ),
                            in_ap=f_in[w : w + WINDOW, :],
                            idxs_ap=it[:, k * IW : (k + 1) * IW],
                            num_idxs=1024,
                            num_idxs_reg=1024,
                            elem_size=D,
                            queue_num=k % 4,
                        )
                    r = sp.tile([128, D], mybir.dt.float32, tag=f"red{h}")
                    nc.vector.tensor_reduce(
                        out=r[:],
                        in_=g[:].rearrange("p (t d) -> p d t", d=D),
                        axis=mybir.AxisListType.X,
                        op=mybir.AluOpType.max,
                    )
                    halves.append(r)
                o = sp.tile([128, D], mybir.dt.float32, tag="out")
                nc.vector.tensor_tensor(
                    out=o[:],
                    in0=halves[0][:],
                    in1=halves[1][:],
                    op=mybir.AluOpType.max,
                )
                nc.sync.dma_start(out=p_out[b * 128 : (b + 1) * 128, :], in_=o[:])

    if not nc.is_finalized():
        nc.finalize()
    return nc


def kernel(features, segment_ids, num_clusters):
    global _last_results
    from concourse.bass_utils import run_bass_kernel_spmd

    features = np.ascontiguousarray(np.asarray(features, dtype=np.float32))
    ids = np.asarray(segment_ids).astype(np.int64)
    nclusters = int(num_clusters)
    assert features.shape == (N_POINTS, D), features.shape
    assert ids.shape == (N_POINTS,)
    assert nclusters == N_CLUSTERS

    # --- host index prep -------------------------------------------------
    gstart = np.searchsorted(ids, np.arange(nclusters), side="left")
    gend = np.searchsorted(ids, np.arange(nclusters) + 1, side="left")
    gcounts = gend - gstart

    core_meta = []  # (cl_lo, ncl, s[NCL], cnt[NCL]) per core
    for c in range(N_CORES):
        r0, r1 = c * RPC, (c + 1) * RPC
        cl_lo, cl_hi = int(ids[r0]), int(ids[r1 - 1])
        ncl = cl_hi - cl_lo + 1
        assert ncl <= NCL, f"core {c}: {ncl} local clusters > {NCL}"
        s = np.clip(gstart[cl_lo : cl_hi + 1], r0, r1) - r0
        e = np.clip(gend[cl_lo : cl_hi + 1], r0, r1) - r0
        cnt = e - s
        s_pad = np.full(NCL, RPC - 1, dtype=np.int64)
        cnt_pad = np.zeros(NCL, dtype=np.int64)
        s_pad[:ncl] = s
        cnt_pad[:ncl] = cnt
        core_meta.append((cl_lo, ncl, s_pad, cnt_pad))

    L = max(int(m[3].max()) for m in core_meta)
    LH = (L + 1) // 2
    LH = max(8, (LH + 7) // 8 * 8)  # pad slots-per-half to a multiple of 8

    # Per-batch windows, shared across cores.
    windows = []
    for b in range(NBATCH):
        wmin = min(int(m[2][b * 128]) for m in core_meta)
        w = max(0, min(wmin, RPC - WINDOW))
        for m in core_meta:
            s_pad, cnt_pad = m[2], m[3]
            jj = slice(b * 128, (b + 1) * 128)
            last = s_pad[jj] + np.maximum(cnt_pad[jj] - 1, 0)
            active = cnt_pad[jj] > 0
            assert not active.any() or (
                (s_pad[jj][active] >= w).all()
                and (last[active] < w + WINDOW).all()
            ), f"batch {b}: window overflow"
        windows.append(w)

    # Build int16 gather tables: gidx[core][b, h, 128, NIDX//16]
    slots = np.arange(2 * LH)
    gidx_all = []
    for c in range(N_CORES):
        _, _, s_pad, cnt_pad = core_meta[c]
        # rows[j, s] = s_j + min(s, cnt_j - 1)   (cnt 0 -> window base)
        rows = s_pad[:, None] + np.minimum(slots[None, :], np.maximum(cnt_pad - 1, 0)[:, None])
        offs = np.empty((NBATCH, 128, 2 * LH), dtype=np.int64)
        for b in range(NBATCH):
            o = rows[b * 128 : (b + 1) * 128] - windows[b]
            o[cnt_pad[b * 128 : (b + 1) * 128] == 0] = 0
            offs[b] = o
        assert offs.min() >= 0 and offs.max() < WINDOW, (offs.min(), offs.max())
        offs16 = offs.astype(np.int16)
        # half h slots [h*LH,(h+1)*LH); gather j-index = t*128+p -> [j%16, j//16]
        NG = LH // 8
        g = np.empty((NBATCH, 2, 128, NG * 64), dtype=np.int16)
        for b in range(NBATCH):
            for h in range(2):
                A = offs16[b, :, h * LH : (h + 1) * LH]  # [128 p, LH]
                # gather k covers slots [k*8,(k+1)*8); flat j = t*128+p
                A = A.reshape(128, NG, 8).transpose(1, 2, 0).reshape(NG, 1024)
                W = A.reshape(NG, 64, 16).transpose(0, 2, 1)  # wrap [j%16, j//16]
                W = W.transpose(1, 0, 2).reshape(16, NG * 64)
                g[b, h] = np.tile(W, (8, 1))
        gidx_all.append(g)

    # --- build + run ------------------------------------------------------
    nc = _build_program(LH, windows)
    in_maps = [
        {
            "features": features[c * RPC : (c + 1) * RPC],
            "gidx": gidx_all[c],
        }
        for c in range(N_CORES)
    ]
    res = run_bass_kernel_spmd(nc, in_maps, list(range(N_CORES)))
    _last_results = res

    # --- host combine -----------------------------------------------------
    full = np.full((nclusters, D), -np.inf, dtype=np.float32)
    for c in range(N_CORES):
        cl_lo, ncl, _, _ = core_meta[c]
        part = res.results[c]["partial"][:ncl]
        full[cl_lo : cl_lo + ncl] = np.maximum(full[cl_lo : cl_lo + ncl], part)
    full[gcounts == 0] = -np.inf
    return full


# revision 5
# speedup vs baseline: 2.9686x; 1.0567x over previous
"""Trainium2 Bass kernel for sorted segment_max (ClusterPool).

Problem: features [2M, 128] f32, segment_ids [2M] sorted int, num_clusters
10000 -> output [10000, 128] = per-cluster max over rows.

Strategy (8 NeuronCores, SPMD):
  - Shard rows: core c gets rows [c*250k, (c+1)*250k). Sorted ids => each
    core covers a contiguous cluster range (~1252 clusters), padded to 1280
    local clusters = 10 batches x 128.
  - Host precomputes, per core, per batch of 128 clusters, an int16 gather
    index table: cluster p's rows (padded by repeating its first row to a
    fixed slot count L). Indices are relative to a per-batch 32768-row
    window of the core's feature slice (clusters are contiguous, so every
    batch's rows fit in one window; verified at runtime).
  - Device: per batch, dma_gather rows into SBUF [128, L, 128] (cluster p on
    partition p), tensor_reduce max over the slot axis -> [128, 128], store
    to the partial output rows. Two gathers of L/2 slots per batch to halve
    SBUF footprint.
  - Host combines the 8 partial outputs (adjacent cores overlap in at most
    one boundary cluster) with np.maximum.
"""

import os
import sys

import numpy as np

sys.path.insert(0, "/opt/trn_rl_repo")

N_POINTS = 2_000_000
D = 128
N_CLUSTERS = 10_000
N_CORES = 8
RPC = N_POINTS // N_CORES  # rows per core
NCL = 1280  # padded local clusters per core
NBATCH = NCL // 128
WINDOW = 32768  # dma_gather int16 index window (rows)

_last_results = None  # BassKernelResults of the most recent run (for test.py)


def _apply_drain_patch():
    """walrus TPB_CTRL supports a single sync wait; TileContext's tail drain
    accumulates one wait per outstanding proc.  Split them across NOPs."""
    import concourse.mybir as mybir
    import concourse.tile as tile
    from concourse.vector_clock import ScopedClock

    if getattr(tile.TileContext, "_drain_patched", False):
        return

    def _patched(self, tick_clock, wait_clock):
        nc = self.nc
        nop = nc.sync.nop(nofuse=True, hint="tail_drain_waits")
        wait_clock.add_sem_waits(nop.ins, ScopedClock({None: tick_clock.global_clock}))
        si = nop.ins.sync_info
        waits = list(si.on_wait) if si is not None and si.on_wait else []
        if len(waits) > 1:
            si.on_wait = waits[:1]
            for i in range(1, len(waits)):
                extra = nc.sync.nop(nofuse=True, hint=f"tail_drain_waits_{i}")
                if extra.ins.sync_info is None:
                    extra.ins.sync_info = mybir.SyncInfo(
                        on_wait=waits[i : i + 1], on_update=[]
                    )
                else:
                    extra.ins.sync_info.on_wait = waits[i : i + 1]
        nc.sync.drain()
        nc.all_engine_barrier()
        assert self.sems is not None
        popped = nc._tile_sem_poison_stack.pop()
        assert popped is self._sem_poison
        nc.clear_and_free_semaphores(list(self.sems.allocated().values()))
        nc.all_engine_barrier()

    tile.TileContext._drain_and_barrier = _patched
    tile.TileContext._drain_patched = True


def _build_program(LH, windows):
    """Build the SPMD Bass program.  LH = slots per half-gather; windows =
    per-batch window base rows (compile-time constants, shared by all cores)."""
    import concourse.bacc as bacc
    import concourse.mybir as mybir
    import concourse.tile as tile

    _apply_drain_patch()

    NG = LH // 8  # gathers per half-batch (1024 idx each: 8 slots/partition)
    IW = 64  # idx cols per gather (1024/16)

    nc = bacc.Bacc(None, num_swdge_queues=4)
    f_in = nc.dram_tensor("features", [RPC, D], mybir.dt.float32, kind="ExternalInput")
    i_in = nc.dram_tensor(
        "gidx", [NBATCH, 2, 128, NG * IW], mybir.dt.int16, kind="ExternalInput"
    )
    p_out = nc.dram_tensor(
        "partial", [NCL, D], mybir.dt.float32, kind="ExternalOutput"
    )

    with tile.TileContext(nc) as tc:
        with (
            tc.tile_pool(name="gp", bufs=2) as gp,
            tc.tile_pool(name="sp", bufs=3) as sp,
        ):
            for b in range(NBATCH):
                w = windows[b]
                halves = []
                for h in range(2):
                    it = sp.tile([128, NG * IW], mybir.dt.int16, tag="idx")
                    nc.sync.dma_start(out=it[:], in_=i_in[b, h])
                    g = gp.tile([128, LH * D], mybir.dt.float32, tag="gath")
                    for k in range(NG):
                        nc.gpsimd.dma_gather(
                            out_ap=g[:, k * 8 * D : (k + 1) * 8 * D].rearrange(
                                "p (t d) -> p t d", d=D
                            ),
                            in_ap=f_in[w : w + WINDOW, :],
                            idxs_ap=it[:, k * IW : (k + 1) * IW],
                            num_idxs=1024,
                            num_idxs_reg=1024,
                            elem_size=D,
                            queue_num=k % 4,
                            single_packet=False,
                        )
                    r = sp.tile([128, D], mybir.dt.float32, tag=f"red{h}")
                    nc.vector.tensor_reduce(
                        out=r[:],
                        in_=g[:].rearrange("p (t d) -> p d t", d=D),
                        axis=mybir.AxisListType.X,
                        op=mybir.AluOpType.max,
                    )
                    halves.append(r)
                o = sp.tile([128, D], mybir.dt.float32, tag="out")
                nc.vector.tensor_tensor(
                    out=o[:],
                    in0=halves[0][:],
                    in1=halves[1][:],
                    op=mybir.AluOpType.max,
                )
                nc.sync.dma_start(out=p_out[b * 128 : (b + 1) * 128, :], in_=o[:])

    if not nc.is_finalized():
        nc.finalize()
    return nc


def kernel(features, segment_ids, num_clusters):
    global _last_results
    from concourse.bass_utils import run_bass_kernel_spmd

    features = np.ascontiguousarray(np.asarray(features, dtype=np.float32))
    ids = np.asarray(segment_ids).astype(np.int64)
    nclusters = int(num_clusters)
    assert features.shape == (N_POINTS, D), features.shape
    assert ids.shape == (N_POINTS,)
    assert nclusters == N_CLUSTERS

    # --- host index prep -------------------------------------------------
    gstart = np.searchsorted(ids, np.arange(nclusters), side="left")
    gend = np.searchsorted(ids, np.arange(nclusters) + 1, side="left")
    gcounts = gend - gstart

    core_meta = []  # (cl_lo, ncl, s[NCL], cnt[NCL]) per core
    for c in range(N_CORES):
        r0, r1 = c * RPC, (c + 1) * RPC
        cl_lo, cl_hi = int(ids[r0]), int(ids[r1 - 1])
        ncl = cl_hi - cl_lo + 1
        assert ncl <= NCL, f"core {c}: {ncl} local clusters > {NCL}"
        s = np.clip(gstart[cl_lo : cl_hi + 1], r0, r1) - r0
        e = np.clip(gend[cl_lo : cl_hi + 1], r0, r1) - r0
        cnt = e - s
        s_pad = np.full(NCL, RPC - 1, dtype=np.int64)
        cnt_pad = np.zeros(NCL, dtype=np.int64)
        s_pad[:ncl] = s
        cnt_pad[:ncl] = cnt
        core_meta.append((cl_lo, ncl, s_pad, cnt_pad))

    L = max(int(m[3].max()) for m in core_meta)
    LH = (L + 1) // 2
    LH = max(8, (LH + 7) // 8 * 8)  # pad slots-per-half to a multiple of 8

    # Per-batch windows, shared across cores.
    windows = []
    for b in range(NBATCH):
        wmin = min(int(m[2][b * 128]) for m in core_meta)
        w = max(0, min(wmin, RPC - WINDOW))
        for m in core_meta:
            s_pad, cnt_pad = m[2], m[3]
            jj = slice(b * 128, (b + 1) * 128)
            last = s_pad[jj] + np.maximum(cnt_pad[jj] - 1, 0)
            active = cnt_pad[jj] > 0
            assert not active.any() or (
                (s_pad[jj][active] >= w).all()
                and (last[active] < w + WINDOW).all()
            ), f"batch {b}: window overflow"
        windows.append(w)

    # Build int16 gather tables: gidx[core][b, h, 128, NIDX//16]
    slots = np.arange(2 * LH)
    gidx_all = []
    for c in range(N_CORES):
        _, _, s_pad, cnt_pad = core_meta[c]
        # rows[j, s] = s_j + min(s, cnt_j - 1)   (cnt 0 -> window base)
        rows = s_pad[:, None] + np.minimum(slots[None, :], np.maximum(cnt_pad - 1, 0)[:, None])
        offs = np.empty((NBATCH, 128, 2 * LH), dtype=np.int64)
        for b in range(NBATCH):
            o = rows[b * 128 : (b + 1) * 128] - windows[b]
            o[cnt_pad[b * 128 : (b + 1) * 128] == 0] = 0
            offs[b] = o
        assert offs.min() >= 0 and offs.max() < WINDOW, (offs.min(), offs.max())
        offs16 = offs.astype(np.int16)
        # half h slots [h*LH,(h+1)*LH); gather j-index = t*128+p -> [j%16, j//16]
        NG = LH // 8
        g = np.empty((NBATCH, 2, 128, NG * 64), dtype=np.int16)
        for b in range(NBATCH):
            for h in range(2):
                A = offs16[b, :, h * LH : (h + 1) * LH]  # [128 p, LH]
                # gather k covers slots [k*8,(k+1)*8); flat j = t*128+p
                A = A.reshape(128, NG, 8).transpose(1, 2, 0).reshape(NG, 1024)
                W = A.reshape(NG, 64, 16).transpose(0, 2, 1)  # wrap [j%16, j//16]
                W = W.transpose(1, 0, 2).reshape(16, NG * 64)
                g[b, h] = np.tile(W, (8, 1))
        gidx_all.append(g)

    # --- build + run ------------------------------------------------------
    nc = _build_program(LH, windows)
    in_maps = [
        {
            "features": features[c * RPC : (c + 1) * RPC],
            "gidx": gidx_all[c],
        }
        for c in range(N_CORES)
    ]
    res = run_bass_kernel_spmd(nc, in_maps, list(range(N_CORES)))
    _last_results = res

    # --- host combine -----------------------------------------------------
    full = np.full((nclusters, D), -np.inf, dtype=np.float32)
    for c in range(N_CORES):
        cl_lo, ncl, _, _ = core_meta[c]
        part = res.results[c]["partial"][:ncl]
        full[cl_lo : cl_lo + ncl] = np.maximum(full[cl_lo : cl_lo + ncl], part)
    full[gcounts == 0] = -np.inf
    return full
